# Initial kernel scaffold
#
"""Trainium2 Bass kernel for the GQA attention layer (B=2, S=2048, HID=2048,
H=16, KVH=4, D=128, causal + RoPE).

Sharding: 8 cores = 2 (batch) x 4 (tensor-parallel over heads).
Core c handles batch b=c//4 and head group tp=c%4 (4 q-heads, 1 kv-head).
Wo is row-sharded; the 4 TP partial outputs per batch are summed on host.

Matmuls run as float32r (full-rate fp32 PE mode). Biases bq/bk/bv are zero in
the problem spec and are skipped. The attention mask is the causal tril by
construction; causality is hardcoded (upper-triangle tiles never computed).
"""
import os
import sys

for p in ("/opt/trn_rl_repo", "/root/.axon_site/_ro/trn_rl_repo"):
    if os.path.isdir(p) and p not in sys.path:
        sys.path.insert(0, p)

import numpy as np

import concourse.bass as bass
import concourse.mybir as mybir
from concourse import bacc
from concourse.tile import TileContext
from concourse.bass_utils import run_bass_kernel_spmd

B, S, HID = 2, 2048, 2048
H, KVH, D = 16, 4, 128
P = 128
TP = 4                      # tensor-parallel ways
HL = H // TP                # q-heads per core (4)
QC = 256                    # q/s chunk size
NCH = S // QC
KO = HID // P               # 16 contraction chunks
SCALE = 1.0 / np.sqrt(D)

F32 = mybir.dt.float32
DT = mybir.dt.float32r      # matmul fast path


def _build_program():
    nc = bacc.Bacc(None, target_bir_lowering=False)

    xb = nc.declare_dram_parameter("xb", [S, HID], DT, isOutput=False)
    wq = nc.declare_dram_parameter("wq", [HID, HL * D], DT, isOutput=False)
    wk = nc.declare_dram_parameter("wk", [HID, D], DT, isOutput=False)
    wv = nc.declare_dram_parameter("wv", [HID, D], DT, isOutput=False)
    wo = nc.declare_dram_parameter("wo", [HL * D, HID], DT, isOutput=False)
    cosm = nc.declare_dram_parameter("cosm", [P, S], DT, isOutput=False)
    sinm = nc.declare_dram_parameter("sinm", [P, S], DT, isOutput=False)
    pswap = nc.declare_dram_parameter("pswap", [P, P], DT, isOutput=False)
    ident = nc.declare_dram_parameter("ident", [P, P], DT, isOutput=False)
    maskc = nc.declare_dram_parameter("maskc", [P, 2, QC], DT, isOutput=False)
    onesm = nc.declare_dram_parameter("onesm", [P, P], DT, isOutput=False)
    out = nc.declare_dram_parameter("out", [S, HID], F32, isOutput=True)


    with TileContext(nc) as tc:
        with (
            tc.tile_pool(name="consts", bufs=1) as consts,
            tc.tile_pool(name="weights", bufs=1) as weights,
            tc.tile_pool(name="persist", bufs=1) as persist,
            tc.tile_pool(name="xload", bufs=1) as xload,
            tc.tile_pool(name="xtp", bufs=1) as xtp,
            tc.tile_pool(name="qop", bufs=1) as qop,
            tc.tile_pool(name="work", bufs=2) as work,
            tc.tile_pool(name="expp", bufs=3) as expp,
            tc.tile_pool(name="outp", bufs=2) as outp,
            tc.tile_pool(name="ps256", bufs=3, space="PSUM") as ps256,
            tc.tile_pool(name="psO", bufs=1, space="PSUM") as psO,
            tc.tile_pool(name="psD", bufs=1, space="PSUM") as psD,
            tc.tile_pool(name="ps128", bufs=3, space="PSUM") as ps128,
        ):
            # ---- constants / weights resident in SBUF ----
            cos_sb = consts.tile([P, S], DT)
            sin_sb = consts.tile([P, S], DT)
            pswap_sb = consts.tile([P, P], DT)
            ident_sb = consts.tile([P, P], DT)
            mask_sb = consts.tile([P, 2, QC], DT)
            ones_sb = consts.tile([P, P], DT)
            nc.sync.dma_start(ident_sb[:], ident[:])

            wq_sb = weights.tile([P, KO, HL * D], DT)
            wk_sb = weights.tile([P, KO, D], DT)
            wv_sb = weights.tile([P, KO, D], DT)
            wo_sb = weights.tile([P, HL, HID], DT)
            wq_r = wq.rearrange("(ko p) m -> p ko m", p=P)
            for kg in range(4):
                nc.gpsimd.dma_start(wq_sb[:, 4 * kg : 4 * (kg + 1), :], wq_r[:, 4 * kg : 4 * (kg + 1), :])
            nc.gpsimd.dma_start(wk_sb[:], wk.rearrange("(ko p) m -> p ko m", p=P))
            nc.gpsimd.dma_start(wv_sb[:], wv.rearrange("(ko p) m -> p ko m", p=P))
            nc.gpsimd.dma_start(pswap_sb[:], pswap[:])
            nc.gpsimd.dma_start(cos_sb[:], cosm[:])
            nc.gpsimd.dma_start(sin_sb[:], sinm[:])
            nc.gpsimd.dma_start(mask_sb[:], maskc[:])
            nc.gpsimd.dma_start(ones_sb[:], onesm[:])
            wo_r = wo.rearrange("(hk p) n -> p hk n", p=P)
            for ng in range(4):
                nc.gpsimd.dma_start(wo_sb[:, :, 512 * ng : 512 * (ng + 1)], wo_r[:, :, 512 * ng : 512 * (ng + 1)])

            # K^T [d, s] and V [s-part, kt, d] accumulate across chunks
            kT_sb = persist.tile([P, S], DT)
            v_sb = persist.tile([P, S // P, D], DT)

            for j in range(NCH):
                s0 = j * QC
                # ---- x^T via PE transposes (128x128), copyback on DVE/ACT ----
                xT_sb = xtp.tile([P, KO, QC], DT)
                for st in range(QC // P):
                    x_sb = xload.tile([P, HID], DT, tag="xrow")
                    nc.sync.dma_start(x_sb[:], xb[s0 + st * P : s0 + (st + 1) * P, :])
                    for ko in range(KO):
                        pst = ps128.tile([P, P], DT, tag="t")
                        nc.tensor.transpose(pst[:], x_sb[:, ko * P : (ko + 1) * P], ident_sb[:])
                        if ko % 2 == 0:
                            nc.vector.tensor_copy(xT_sb[:, ko, st * P : (st + 1) * P], pst[:])
                        else:
                            nc.scalar.copy(xT_sb[:, ko, st * P : (st + 1) * P], pst[:])

                # ---- Q projection + rope (4 heads) ----
                qT_sb = qop.tile([P, HL, QC], DT)
                for h in range(HL):
                    psq = ps256.tile([P, QC], F32, tag="s")
                    for ko in range(KO):
                        nc.tensor.matmul(
                            psq[:], wq_sb[:, ko, h * D : (h + 1) * D], xT_sb[:, ko, :],
                            start=(ko == 0), stop=(ko == KO - 1),
                        )
                    q_raw = work.tile([P, QC], DT)
                    nc.vector.tensor_copy(q_raw[:], psq[:])
                    pssw = ps256.tile([P, QC], F32, tag="s")
                    nc.tensor.matmul(pssw[:], pswap_sb[:], q_raw[:], start=True, stop=True)
                    t1 = work.tile([P, QC], DT, tag="ropet")
                    t2 = work.tile([P, QC], DT, tag="ropet")
                    nc.vector.tensor_tensor(t1[:], q_raw[:], cos_sb[:, s0 : s0 + QC], mybir.AluOpType.mult)
                    nc.vector.tensor_tensor(t2[:], pssw[:], sin_sb[:, s0 : s0 + QC], mybir.AluOpType.mult)
                    nc.vector.tensor_add(qT_sb[:, h, :], t1[:], t2[:])

                # ---- K projection + rope ----
                psk = ps256.tile([P, QC], F32, tag="s")
                for ko in range(KO):
                    nc.tensor.matmul(
                        psk[:], wk_sb[:, ko, :], xT_sb[:, ko, :],
                        start=(ko == 0), stop=(ko == KO - 1),
                    )
                k_raw = work.tile([P, QC], DT)
                nc.vector.tensor_copy(k_raw[:], psk[:])
                pskw = ps256.tile([P, QC], F32, tag="s")
                nc.tensor.matmul(pskw[:], pswap_sb[:], k_raw[:], start=True, stop=True)
                t1k = work.tile([P, QC], DT, tag="ropet")
                t2k = work.tile([P, QC], DT, tag="ropet")
                nc.vector.tensor_tensor(t1k[:], k_raw[:], cos_sb[:, s0 : s0 + QC], mybir.AluOpType.mult)
                nc.vector.tensor_tensor(t2k[:], pskw[:], sin_sb[:, s0 : s0 + QC], mybir.AluOpType.mult)
                nc.vector.tensor_add(kT_sb[:, s0 : s0 + QC], t1k[:], t2k[:])

                # ---- V projection as V^T [d, s], then PE-transpose to [s, d] ----
                psv = ps256.tile([P, QC], F32, tag="s")
                for ko in range(KO):
                    nc.tensor.matmul(
                        psv[:], wv_sb[:, ko, :], xT_sb[:, ko, :],
                        start=(ko == 0), stop=(ko == KO - 1),
                    )
                vT_sb = work.tile([P, QC], DT, tag="vT")
                nc.vector.tensor_copy(vT_sb[:], psv[:])
                for st in range(QC // P):
                    psvt = ps128.tile([P, P], DT, tag="t")
                    nc.tensor.transpose(psvt[:], vT_sb[:, st * P : (st + 1) * P], ident_sb[:])
                    nc.vector.tensor_copy(v_sb[:, 2 * j + st, :], psvt[:])

                # ---- attention for this q-chunk (scores transposed [k, q]) ----
                nkt = 2 * j + 2
                oT_sb = qop.tile([P, HL, QC], DT, name="oT_sb")
                for h in range(HL):
                    pso = psO.tile([P, QC], F32)
                    psden = psD.tile([P, QC], F32)
                    for kt in range(nkt):
                        pss = ps256.tile([P, QC], F32, tag="s")
                        nc.tensor.matmul(
                            pss[:], kT_sb[:, kt * P : (kt + 1) * P], qT_sb[:, h, :],
                            start=True, stop=True,
                        )
                        ex = expp.tile([P, QC], DT)
                        nc.scalar.activation(ex[:], pss[:], mybir.ActivationFunctionType.Exp, scale=float(SCALE))
                        if kt >= 2 * j:
                            nc.vector.tensor_tensor(ex[:], ex[:], mask_sb[:, kt - 2 * j, :], mybir.AluOpType.mult)
                        nc.tensor.matmul(psden[:], ones_sb[:], ex[:], start=(kt == 0), stop=(kt == nkt - 1))
                        nc.tensor.matmul(pso[:], v_sb[:, kt, :], ex[:], start=(kt == 0), stop=(kt == nkt - 1))
                    rb_sb = work.tile([P, QC], DT, tag="rbcast")
                    with nc.allow_low_precision(reason="float32r is fp32 bits"):
                        nc.vector.reciprocal(rb_sb[:], psden[:])
                    nc.vector.tensor_tensor(oT_sb[:, h, :], pso[:], rb_sb[:], mybir.AluOpType.mult)

                # ---- Wo partial for this chunk: out[s, hid] ----
                for st in range(QC // P):
                    ob = outp.tile([P, HID], F32)
                    for nj in range(HID // 256):
                        pswt = ps256.tile([P, QC], F32, tag="s")
                        for hk in range(HL):
                            nc.tensor.matmul(
                                pswt[:],
                                oT_sb[:, hk, st * P : (st + 1) * P],
                                wo_sb[:, hk, nj * 256 : (nj + 1) * 256],
                                start=(hk == 0), stop=(hk == HL - 1),
                            )
                        nc.vector.tensor_copy(ob[:, nj * 256 : (nj + 1) * 256], pswt[:])
                    nc.sync.dma_start(out[s0 + st * P : s0 + (st + 1) * P, :], ob[:])

    nc.compile()
    return nc


_NC_CACHE = None


def _host_constants(rope_cache):
    cos = np.repeat(rope_cache[:, :, 0].T, 2, axis=0).astype(np.float32)  # [128, S]
    sin_base = np.repeat(rope_cache[:, :, 1].T, 2, axis=0).astype(np.float32)
    sign = np.where(np.arange(P) % 2 == 0, -1.0, 1.0).astype(np.float32)
    sin = sin_base * sign[:, None]
    pswap = np.zeros((P, P), np.float32)
    idx = np.arange(P)
    pswap[idx, idx ^ 1] = 1.0
    ident = np.eye(P, dtype=np.float32)
    kk = np.arange(P)[:, None, None]
    oo = np.arange(2)[None, :, None]
    qq = np.arange(QC)[None, None, :]
    maskc = (qq >= 128 * oo + kk).astype(np.float32)
    ones = np.ones((P, P), np.float32)
    return cos, sin, pswap, ident, maskc, ones


def _build_in_maps(inputs):
    x = np.asarray(inputs["x"], np.float32)
    rope_cache = np.asarray(inputs["rope_cache"], np.float32)
    Wq = np.asarray(inputs["Wq"], np.float32)
    Wk = np.asarray(inputs["Wk"], np.float32)
    Wv = np.asarray(inputs["Wv"], np.float32)
    Wo = np.asarray(inputs["Wo"], np.float32)

    cos, sin, pswap, ident, maskc, ones = _host_constants(rope_cache)

    in_maps = []
    for core in range(8):
        b, tp = divmod(core, 4)
        in_maps.append({
            "xb": np.ascontiguousarray(x[b]),
            "wq": np.ascontiguousarray(Wq[:, tp * HL * D : (tp + 1) * HL * D]),
            "wk": np.ascontiguousarray(Wk[:, tp * D : (tp + 1) * D]),
            "wv": np.ascontiguousarray(Wv[:, tp * D : (tp + 1) * D]),
            "wo": np.ascontiguousarray(Wo[tp * HL * D : (tp + 1) * HL * D, :]),
            "cosm": cos, "sinm": sin, "pswap": pswap, "ident": ident, "maskc": maskc,
            "onesm": ones,
        })
    return in_maps


def kernel(x, attention_mask, rope_cache, Wq, bq, Wk, bk, Wv, bv, Wo):
    global _NC_CACHE
    in_maps = _build_in_maps({"x": x, "rope_cache": rope_cache,
                              "Wq": Wq, "Wk": Wk, "Wv": Wv, "Wo": Wo})

    if _NC_CACHE is None:
        _NC_CACHE = _build_program()
    r = run_bass_kernel_spmd(_NC_CACHE, in_maps, list(range(8)))

    outf = np.zeros((B, S, HID), np.float32)
    for core in range(8):
        b = core // 4
        outf[b] += r.results[core]["out"]
    return outf



# revision 11
# speedup vs baseline: 1.1603x; 1.1603x over previous
"""Trainium2 Bass kernel for the GQA attention layer (B=2, S=2048, HID=2048,
H=16, KVH=4, D=128, causal + RoPE).

Sharding: 8 cores = 2 (batch) x 4 (tensor-parallel over heads).
Core c handles batch b=c//4 and head group tp=c%4 (4 q-heads, 1 kv-head).
Wo is row-sharded; the 4 TP partial outputs per batch are summed on host.

v2: x is pre-transposed on the host (xT [HID, S] in DRAM) so the kernel
never runs PE transposes for the projections; all matmuls use 512-wide
moving operands (fp32 max); causal diagonal tiles compute only the live
q-range into zero-prefixed exp buffers; reciprocal_approx_fast for the
softmax denominator.

Matmuls run as float32r (full-rate fp32 PE mode). Biases bq/bk/bv are zero in
the problem spec and are skipped. The attention mask is the causal tril by
construction; causality is hardcoded (upper-triangle tiles never computed).
"""
import os
import sys

for p in ("/opt/trn_rl_repo", "/root/.axon_site/_ro/trn_rl_repo"):
    if os.path.isdir(p) and p not in sys.path:
        sys.path.insert(0, p)

import numpy as np

import concourse.bass as bass
import concourse.mybir as mybir
from concourse import bacc
from concourse.tile import TileContext
from concourse.bass_utils import run_bass_kernel_spmd

B, S, HID = 2, 2048, 2048
H, KVH, D = 16, 4, 128
P = 128
TP = 4                      # tensor-parallel ways
HL = H // TP                # q-heads per core (4)
QC = 512                    # q/s chunk size (fp32 moving-operand max)
NCH = S // QC               # 4 chunks
KO = HID // P               # 16 contraction chunks
DIAG = QC // P              # 4 diagonal 128-k tiles per chunk
SCALE = 1.0 / np.sqrt(D)

F32 = mybir.dt.float32
DT = mybir.dt.float32r      # matmul fast path


def _build_program():
    nc = bacc.Bacc(None, target_bir_lowering=False)

    xT = nc.declare_dram_parameter("xT", [HID, S], DT, isOutput=False)
    wq = nc.declare_dram_parameter("wq", [HID, HL * D], DT, isOutput=False)
    wk = nc.declare_dram_parameter("wk", [HID, D], DT, isOutput=False)
    wv = nc.declare_dram_parameter("wv", [HID, D], DT, isOutput=False)
    wo = nc.declare_dram_parameter("wo", [HL * D, HID], DT, isOutput=False)
    cosm = nc.declare_dram_parameter("cosm", [P, S], DT, isOutput=False)
    sinm = nc.declare_dram_parameter("sinm", [P, S], DT, isOutput=False)
    pswap = nc.declare_dram_parameter("pswap", [P, P], DT, isOutput=False)
    ident = nc.declare_dram_parameter("ident", [P, P], DT, isOutput=False)
    trim = nc.declare_dram_parameter("trim", [P, P], DT, isOutput=False)
    onesm = nc.declare_dram_parameter("onesm", [P, P], DT, isOutput=False)
    zerom = nc.declare_dram_parameter("zerom", [P, QC], DT, isOutput=False)
    out = nc.declare_dram_parameter("out", [S, HID], F32, isOutput=True)

    with TileContext(nc) as tc:
        with (
            tc.tile_pool(name="consts", bufs=1) as consts,
            tc.tile_pool(name="weights", bufs=1) as weights,
            tc.tile_pool(name="persist", bufs=1) as persist,
            tc.tile_pool(name="xload", bufs=1) as xload,
            tc.tile_pool(name="cspool", bufs=1) as cspool,
            tc.tile_pool(name="qop", bufs=2) as qop,
            tc.tile_pool(name="work", bufs=2) as work,
            tc.tile_pool(name="expp", bufs=4) as expp,
            tc.tile_pool(name="outp", bufs=2) as outp,
            tc.tile_pool(name="ps3", bufs=4, space="PSUM") as ps3,
            tc.tile_pool(name="psAcc", bufs=2, space="PSUM") as psAcc,
        ):
            # ---- constants ----
            pswap_sb = consts.tile([P, P], DT)
            ident_sb = consts.tile([P, P], DT)
            tri_sb = consts.tile([P, P], DT)
            ones_sb = consts.tile([P, P], DT)
            nc.sync.dma_start(ident_sb[:], ident[:])
            nc.gpsimd.dma_start(pswap_sb[:], pswap[:])
            nc.gpsimd.dma_start(tri_sb[:], trim[:])
            nc.gpsimd.dma_start(ones_sb[:], onesm[:])

            # ---- PE warm-up: keep HAM busy while the big weight DMAs land ----
            for _ in range(40):
                pwu = ps3.tile([P, P], DT, tag="s")
                nc.tensor.transpose(pwu[:], ident_sb[:], ident_sb[:])

            # ---- weights resident in SBUF ----
            wq_sb = weights.tile([P, KO, HL * D], DT)
            wk_sb = weights.tile([P, KO, D], DT)
            wv_sb = weights.tile([P, KO, D], DT)
            wo_sb = weights.tile([P, HL, HID], DT)
            nc.gpsimd.dma_start(wk_sb[:], wk.rearrange("(ko p) m -> p ko m", p=P))
            nc.gpsimd.dma_start(wv_sb[:], wv.rearrange("(ko p) m -> p ko m", p=P))
            wq_r = wq.rearrange("(ko p) m -> p ko m", p=P)
            for kg in range(4):
                nc.gpsimd.dma_start(wq_sb[:, 4 * kg : 4 * (kg + 1), :], wq_r[:, 4 * kg : 4 * (kg + 1), :])
            wo_r = wo.rearrange("(hk p) n -> p hk n", p=P)
            for ng in range(4):
                nc.gpsimd.dma_start(wo_sb[:, :, 512 * ng : 512 * (ng + 1)], wo_r[:, :, 512 * ng : 512 * (ng + 1)])

            # K^T [d, s] and V [s-part, kt, d] accumulate across chunks
            kT_sb = persist.tile([P, S], DT)
            v_sb = persist.tile([P, S // P, D], DT)

            # zero-prefixed exp buffers for the 4 diagonal tile shapes
            exd_sb = persist.tile([P, DIAG, QC], DT)
            for c in range(DIAG):
                nc.gpsimd.dma_start(exd_sb[:, c, :], zerom[:])

            xT_r = xT.rearrange("(ko p) s -> p ko s", p=P)

            for j in range(NCH):
                s0 = j * QC
                nkt = DIAG * j + DIAG  # k-tiles this chunk (incl. 4 diagonal)

                # ---- x^T chunk: straight DMA (host pre-transposed) ----
                xT_sb = xload.tile([P, KO, QC], DT)
                for kg in range(8):
                    eng = nc.sync if kg % 2 == 0 else nc.gpsimd
                    eng.dma_start(
                        xT_sb[:, 2 * kg : 2 * (kg + 1), :],
                        xT_r[:, 2 * kg : 2 * (kg + 1), s0 : s0 + QC],
                    )
                cos_sb = cspool.tile([P, QC], DT, tag="cos")
                sin_sb = cspool.tile([P, QC], DT, tag="sin")
                nc.sync.dma_start(cos_sb[:], cosm[:, s0 : s0 + QC])
                nc.sync.dma_start(sin_sb[:], sinm[:, s0 : s0 + QC])

                # ---- K projection + rope ----
                psk = ps3.tile([P, QC], F32, tag="s")
                for ko in range(KO):
                    nc.tensor.matmul(
                        psk[:], wk_sb[:, ko, :], xT_sb[:, ko, :],
                        start=(ko == 0), stop=(ko == KO - 1),
                    )
                k_raw = work.tile([P, QC], DT, tag="qraw")
                nc.scalar.copy(k_raw[:], psk[:])
                t1k = work.tile([P, QC], DT, tag="t1")
                nc.vector.tensor_tensor(t1k[:], psk[:], cos_sb[:], mybir.AluOpType.mult)
                pskw = ps3.tile([P, QC], F32, tag="s")
                nc.tensor.matmul(pskw[:], pswap_sb[:], k_raw[:], start=True, stop=True)
                t2k = work.tile([P, QC], DT, tag="t2")
                nc.vector.tensor_tensor(t2k[:], pskw[:], sin_sb[:], mybir.AluOpType.mult)
                nc.vector.tensor_add(kT_sb[:, s0 : s0 + QC], t1k[:], t2k[:])

                # ---- V projection as V^T [d, s], then PE-transpose to [s, d] ----
                psv = ps3.tile([P, QC], F32, tag="s")
                for ko in range(KO):
                    nc.tensor.matmul(
                        psv[:], wv_sb[:, ko, :], xT_sb[:, ko, :],
                        start=(ko == 0), stop=(ko == KO - 1),
                    )
                vT_sb = work.tile([P, QC], DT, tag="qraw")
                nc.scalar.copy(vT_sb[:], psv[:])
                for st in range(QC // P):
                    psvt = ps3.tile([P, P], DT, tag="s")
                    nc.tensor.transpose(psvt[:], vT_sb[:, st * P : (st + 1) * P], ident_sb[:])
                    nc.vector.tensor_copy(v_sb[:, DIAG * j + st, :], psvt[:])

                # ---- Q projection + rope (4 heads) ----
                qT_sb = qop.tile([P, HL, QC], DT, tag="qT")
                for h in range(HL):
                    psq = ps3.tile([P, QC], F32, tag="s")
                    for ko in range(KO):
                        nc.tensor.matmul(
                            psq[:], wq_sb[:, ko, h * D : (h + 1) * D], xT_sb[:, ko, :],
                            start=(ko == 0), stop=(ko == KO - 1),
                        )
                    q_raw = work.tile([P, QC], DT, tag="qraw")
                    nc.scalar.copy(q_raw[:], psq[:])
                    t1 = work.tile([P, QC], DT, tag="t1")
                    nc.vector.tensor_tensor(t1[:], psq[:], cos_sb[:], mybir.AluOpType.mult)
                    pssw = ps3.tile([P, QC], F32, tag="s")
                    nc.tensor.matmul(pssw[:], pswap_sb[:], q_raw[:], start=True, stop=True)
                    t2 = work.tile([P, QC], DT, tag="t2")
                    nc.vector.tensor_tensor(t2[:], pssw[:], sin_sb[:], mybir.AluOpType.mult)
                    nc.vector.tensor_add(qT_sb[:, h, :], t1[:], t2[:])

                # ---- attention for this q-chunk (scores transposed [k, q]) ----
                oT_sb = qop.tile([P, HL, QC], DT, tag="oT")
                for h in range(HL):
                    psO = psAcc.tile([P, QC], F32, tag="O")
                    psD = psAcc.tile([P, QC], F32, tag="D")
                    for kt in range(nkt):
                        c = kt - DIAG * j  # >= 0 on the diagonal band
                        if c < 0:
                            pss = ps3.tile([P, QC], F32, tag="s")
                            nc.tensor.matmul(
                                pss[:], kT_sb[:, kt * P : (kt + 1) * P], qT_sb[:, h, :],
                                start=True, stop=True,
                            )
                            ex = expp.tile([P, QC], DT, tag="ex")
                            nc.scalar.activation(ex[:], pss[:], mybir.ActivationFunctionType.Exp,
                                                 scale=float(SCALE))
                            exv = ex[:]
                        else:
                            q0 = c * P  # live q-range is [q0, QC)
                            pss = ps3.tile([P, QC], F32, tag="s")
                            nc.tensor.matmul(
                                pss[:, q0:], kT_sb[:, kt * P : (kt + 1) * P],
                                qT_sb[:, h, q0:], start=True, stop=True,
                            )
                            nc.scalar.activation(exd_sb[:, c, q0:], pss[:, q0:],
                                                 mybir.ActivationFunctionType.Exp,
                                                 scale=float(SCALE))
                            nc.vector.tensor_tensor(
                                exd_sb[:, c, q0 : q0 + P], exd_sb[:, c, q0 : q0 + P],
                                tri_sb[:], mybir.AluOpType.mult,
                            )
                            exv = exd_sb[:, c, :]
                        nc.tensor.matmul(psD[:], ones_sb[:], exv,
                                         start=(kt == 0), stop=(kt == nkt - 1))
                        nc.tensor.matmul(psO[:], v_sb[:, kt, :], exv,
                                         start=(kt == 0), stop=(kt == nkt - 1))
                    rb = work.tile([P, QC], F32, tag="rb")
                    nc.vector.reciprocal_approx_fast(rb[:], psD[:])
                    nc.vector.tensor_tensor(oT_sb[:, h, :], psO[:], rb[:], mybir.AluOpType.mult)

                # ---- Wo partial for this chunk: out[s, hid] ----
                for st in range(QC // P):
                    for nj in range(HID // 512):
                        pswt = ps3.tile([P, QC], F32, tag="s")
                        for hk in range(HL):
                            nc.tensor.matmul(
                                pswt[:],
                                oT_sb[:, hk, st * P : (st + 1) * P],
                                wo_sb[:, hk, nj * 512 : (nj + 1) * 512],
                                start=(hk == 0), stop=(hk == HL - 1),
                            )
                        ob = outp.tile([P, 512], F32, tag="ob")
                        nc.vector.tensor_copy(ob[:], pswt[:])
                        nc.sync.dma_start(
                            out[s0 + st * P : s0 + (st + 1) * P, nj * 512 : (nj + 1) * 512],
                            ob[:],
                        )

    nc.compile()
    return nc


_NC_CACHE = None


def _host_constants(rope_cache):
    cos = np.repeat(rope_cache[:, :, 0].T, 2, axis=0).astype(np.float32)  # [128, S]
    sin_base = np.repeat(rope_cache[:, :, 1].T, 2, axis=0).astype(np.float32)
    sign = np.where(np.arange(P) % 2 == 0, -1.0, 1.0).astype(np.float32)
    sin = sin_base * sign[:, None]
    pswap = np.zeros((P, P), np.float32)
    idx = np.arange(P)
    pswap[idx, idx ^ 1] = 1.0
    ident = np.eye(P, dtype=np.float32)
    tri = (np.arange(P)[None, :] >= np.arange(P)[:, None]).astype(np.float32)  # [k, q]
    ones = np.ones((P, P), np.float32)
    return cos, sin, pswap, ident, tri, ones


def _build_in_maps(inputs):
    x = np.asarray(inputs["x"], np.float32)
    rope_cache = np.asarray(inputs["rope_cache"], np.float32)
    Wq = np.asarray(inputs["Wq"], np.float32)
    Wk = np.asarray(inputs["Wk"], np.float32)
    Wv = np.asarray(inputs["Wv"], np.float32)
    Wo = np.asarray(inputs["Wo"], np.float32)

    cos, sin, pswap, ident, tri, ones = _host_constants(rope_cache)

    in_maps = []
    for core in range(8):
        b, tp = divmod(core, 4)
        in_maps.append({
            "xT": np.ascontiguousarray(x[b].T),
            "wq": np.ascontiguousarray(Wq[:, tp * HL * D : (tp + 1) * HL * D]),
            "wk": np.ascontiguousarray(Wk[:, tp * D : (tp + 1) * D]),
            "wv": np.ascontiguousarray(Wv[:, tp * D : (tp + 1) * D]),
            "wo": np.ascontiguousarray(Wo[tp * HL * D : (tp + 1) * HL * D, :]),
            "cosm": cos, "sinm": sin, "pswap": pswap, "ident": ident, "trim": tri,
            "onesm": ones, "zerom": np.zeros((P, QC), np.float32),
        })
    return in_maps


def kernel(x, attention_mask, rope_cache, Wq, bq, Wk, bk, Wv, bv, Wo):
    global _NC_CACHE
    in_maps = _build_in_maps({"x": x, "rope_cache": rope_cache,
                              "Wq": Wq, "Wk": Wk, "Wv": Wv, "Wo": Wo})

    if _NC_CACHE is None:
        _NC_CACHE = _build_program()
    r = run_bass_kernel_spmd(_NC_CACHE, in_maps, list(range(8)))

    outf = np.zeros((B, S, HID), np.float32)
    for core in range(8):
        b = core // 4
        outf[b] += r.results[core]["out"]
    return outf


# revision 19
# speedup vs baseline: 1.4694x; 1.2664x over previous
"""Trainium2 Bass kernel for the GQA attention layer (B=2, S=2048, HID=2048,
H=16, KVH=4, D=128, causal + RoPE).

Sharding: 8 cores = 2 (batch) x 4 (tensor-parallel over heads).
Core c handles batch b=c//4 and head group tp=c%4 (4 q-heads, 1 kv-head).
Wo is row-sharded; the 4 TP partial outputs per batch are summed on host.

v7: x is pre-transposed on the host (xT [HID, S] in DRAM) so the kernel
never runs PE transposes for the projections; all matmuls run as float32r
(full-rate fp32 PE mode) with 512-wide moving operands; causal diagonal tiles compute only the live q-range
into zero-prefixed exp buffers; reciprocal_approx_fast for the softmax
denominator. Biases bq/bk/bv are zero in the problem spec and are skipped.
The attention mask is the causal tril by construction; causality is hardcoded.
"""
import os
import sys

for p in ("/opt/trn_rl_repo", "/root/.axon_site/_ro/trn_rl_repo"):
    if os.path.isdir(p) and p not in sys.path:
        sys.path.insert(0, p)

import numpy as np

import concourse.bass as bass
import concourse.mybir as mybir
from concourse import bacc
from concourse.tile import TileContext
from concourse.bass_utils import run_bass_kernel_spmd

B, S, HID = 2, 2048, 2048
H, KVH, D = 16, 4, 128
P = 128
TP = 4                      # tensor-parallel ways
HL = H // TP                # q-heads per core (4)
QC = 512                    # q/s chunk size (fp32 moving-operand max)
NCH = S // QC               # 4 chunks
KO = HID // P               # 16 contraction chunks
DIAG = QC // P              # 4 diagonal 128-k tiles per chunk
SCALE = 1.0 / np.sqrt(D)

F32 = mybir.dt.float32
DT = mybir.dt.float32r      # attention matmul fast path


def _build_program():
    nc = bacc.Bacc(None, target_bir_lowering=False)

    xT = nc.declare_dram_parameter("xT", [HID, S], DT, isOutput=False)
    wq = nc.declare_dram_parameter("wq", [HID, HL * D], DT, isOutput=False)
    wk = nc.declare_dram_parameter("wk", [HID, D], DT, isOutput=False)
    wv = nc.declare_dram_parameter("wv", [HID, D], DT, isOutput=False)
    wo = nc.declare_dram_parameter("wo", [HL * D, HID], DT, isOutput=False)
    cosm = nc.declare_dram_parameter("cosm", [P, S], DT, isOutput=False)
    sinm = nc.declare_dram_parameter("sinm", [P, S], DT, isOutput=False)
    pswap = nc.declare_dram_parameter("pswap", [P, P], DT, isOutput=False)
    ident = nc.declare_dram_parameter("ident", [P, P], DT, isOutput=False)
    trim = nc.declare_dram_parameter("trim", [P, P], DT, isOutput=False)
    onesm = nc.declare_dram_parameter("onesm", [P, P], DT, isOutput=False)
    zerom = nc.declare_dram_parameter("zerom", [P, QC], DT, isOutput=False)
    out = nc.declare_dram_parameter("out", [S, HID], F32, isOutput=True)

    with TileContext(nc) as tc:
        with (
            tc.tile_pool(name="consts", bufs=1) as consts,
            tc.tile_pool(name="weights", bufs=1) as weights,
            tc.tile_pool(name="persist", bufs=1) as persist,
            tc.tile_pool(name="xload", bufs=1) as xload,
            tc.tile_pool(name="cspool", bufs=1) as cspool,
            tc.tile_pool(name="qop", bufs=2) as qop,
            tc.tile_pool(name="work", bufs=2) as work,
            tc.tile_pool(name="expp", bufs=4) as expp,
            tc.tile_pool(name="outp", bufs=3) as outp,
            tc.tile_pool(name="ps3", bufs=4, space="PSUM") as ps3,
            tc.tile_pool(name="psAcc", bufs=2, space="PSUM") as psAcc,
        ):
            # ---- constants ----
            pswap_sb = consts.tile([P, P], DT)
            ident_sb = consts.tile([P, P], DT)
            tri_sb = consts.tile([P, P], DT)
            ones_sb = consts.tile([P, P], DT)
            nc.sync.dma_start(ident_sb[:], ident[:])
            nc.gpsimd.dma_start(pswap_sb[:], pswap[:])
            nc.gpsimd.dma_start(tri_sb[:], trim[:])
            nc.gpsimd.dma_start(ones_sb[:], onesm[:])

            # ---- PE warm-up: keep HAM busy while the big weight DMAs land ----
            for _ in range(64):
                pwu = ps3.tile([P, P], DT, tag="s")
                nc.tensor.transpose(pwu[:], ident_sb[:], ident_sb[:])

            # ---- weights resident in SBUF ----
            wq_sb = weights.tile([P, KO, HL * D], DT)
            wk_sb = weights.tile([P, KO, D], DT)
            wv_sb = weights.tile([P, KO, D], DT)
            wo_sb = weights.tile([P, HL, HID], DT)
            nc.gpsimd.dma_start(wk_sb[:], wk.rearrange("(ko p) m -> p ko m", p=P))
            nc.gpsimd.dma_start(wv_sb[:], wv.rearrange("(ko p) m -> p ko m", p=P))
            wq_r = wq.rearrange("(ko p) m -> p ko m", p=P)
            for kg in range(4):
                nc.gpsimd.dma_start(wq_sb[:, 4 * kg : 4 * (kg + 1), :], wq_r[:, 4 * kg : 4 * (kg + 1), :])
            wo_r = wo.rearrange("(hk p) n -> p hk n", p=P)
            for ng in range(4):
                nc.gpsimd.dma_start(wo_sb[:, :, 512 * ng : 512 * (ng + 1)], wo_r[:, :, 512 * ng : 512 * (ng + 1)])

            # K^T [d, s] and V [s-part, kt, d] accumulate across chunks
            kT_sb = persist.tile([P, S], DT)
            v_sb = persist.tile([P, S // P, D], DT)

            # zero-prefixed exp buffers for the 4 diagonal tile shapes
            exd_sb = persist.tile([P, DIAG, QC], DT)
            for c in range(DIAG):
                nc.gpsimd.dma_start(exd_sb[:, c, :], zerom[:])

            xT_r = xT.rearrange("(ko p) s -> p ko s", p=P)

            for j in range(NCH):
                s0 = j * QC
                nkt = DIAG * j + DIAG  # k-tiles this chunk (incl. 4 diagonal)

                # ---- x^T chunk: straight DMA (host pre-transposed) ----
                xT_sb = xload.tile([P, KO, QC], DT)
                xeng = nc.sync if j == 0 else nc.gpsimd
                for kg in range(4):
                    xeng.dma_start(
                        xT_sb[:, 4 * kg : 4 * (kg + 1), :],
                        xT_r[:, 4 * kg : 4 * (kg + 1), s0 : s0 + QC],
                    )
                cos_sb = cspool.tile([P, QC], DT, tag="cos")
                sin_sb = cspool.tile([P, QC], DT, tag="sin")
                nc.sync.dma_start(cos_sb[:], cosm[:, s0 : s0 + QC])
                nc.sync.dma_start(sin_sb[:], sinm[:, s0 : s0 + QC])

                # ---- K projection + rope ----
                psk = ps3.tile([P, QC], F32, tag="s")
                for ko in range(KO):
                    nc.tensor.matmul(
                        psk[:], wk_sb[:, ko, :], xT_sb[:, ko, :],
                        start=(ko == 0), stop=(ko == KO - 1),
                    )
                k_raw = work.tile([P, QC], DT, tag="qraw")
                nc.scalar.copy(k_raw[:], psk[:])
                t1k = work.tile([P, QC], DT, tag="t1")
                nc.vector.tensor_tensor(t1k[:], k_raw[:], cos_sb[:], mybir.AluOpType.mult)

                # ---- V projection as V^T [d, s], then PE-transpose to [s, d] ----
                psv = ps3.tile([P, QC], F32, tag="s")
                for ko in range(KO):
                    nc.tensor.matmul(
                        psv[:], wv_sb[:, ko, :], xT_sb[:, ko, :],
                        start=(ko == 0), stop=(ko == KO - 1),
                    )
                vT_sb = work.tile([P, QC], DT, tag="qraw")
                nc.scalar.copy(vT_sb[:], psv[:])
                for st in range(QC // P):
                    psvt = ps3.tile([P, P], DT, tag="s")
                    nc.tensor.transpose(psvt[:], vT_sb[:, st * P : (st + 1) * P], ident_sb[:])
                    nc.vector.tensor_copy(v_sb[:, DIAG * j + st, :], psvt[:])

                # deferred K swap (k_raw has long been copied; no PE stall)
                pskw = ps3.tile([P, QC], F32, tag="s")
                nc.tensor.matmul(pskw[:], pswap_sb[:], k_raw[:], start=True, stop=True)
                t2k = work.tile([P, QC], DT, tag="t2")
                nc.vector.tensor_tensor(t2k[:], pskw[:], sin_sb[:], mybir.AluOpType.mult)
                nc.vector.tensor_add(kT_sb[:, s0 : s0 + QC], t1k[:], t2k[:])

                # ---- Q projection + rope (4 heads) ----
                qT_sb = qop.tile([P, HL, QC], DT, tag="qT")

                def emit_qswap(hh, q_raw_h, t1_h):
                    pssw = ps3.tile([P, QC], F32, tag="s")
                    nc.tensor.matmul(pssw[:], pswap_sb[:], q_raw_h[:], start=True, stop=True)
                    t2 = work.tile([P, QC], DT, tag="t2")
                    nc.vector.tensor_tensor(t2[:], pssw[:], sin_sb[:], mybir.AluOpType.mult)
                    nc.vector.tensor_add(qT_sb[:, hh, :], t1_h[:], t2[:])

                pending = None
                for h in range(HL):
                    psq = ps3.tile([P, QC], F32, tag="s")
                    for ko in range(KO):
                        nc.tensor.matmul(
                            psq[:], wq_sb[:, ko, h * D : (h + 1) * D], xT_sb[:, ko, :],
                            start=(ko == 0), stop=(ko == KO - 1),
                        )
                    if pending is not None:
                        emit_qswap(*pending)
                    q_raw = work.tile([P, QC], DT, tag="qraw")
                    nc.scalar.copy(q_raw[:], psq[:])
                    t1 = work.tile([P, QC], DT, tag="t1")
                    nc.vector.tensor_tensor(t1[:], q_raw[:], cos_sb[:], mybir.AluOpType.mult)
                    pending = (h, q_raw, t1)
                emit_qswap(*pending)

                # ---- attention for this q-chunk (scores transposed [k, q]) ----
                oT_sb = qop.tile([P, HL, QC], DT, tag="oT")
                for h in range(HL):
                    psO = psAcc.tile([P, QC], F32, tag="O")
                    psD = psAcc.tile([P, QC], F32, tag="D")
                    for kt in range(nkt):
                        c = kt - DIAG * j  # >= 0 on the diagonal band
                        if c < 0:
                            pss = ps3.tile([P, QC], F32, tag="s")
                            nc.tensor.matmul(
                                pss[:], kT_sb[:, kt * P : (kt + 1) * P], qT_sb[:, h, :],
                                start=True, stop=True,
                            )
                            ex = expp.tile([P, QC], DT, tag="ex")
                            nc.scalar.activation(ex[:], pss[:], mybir.ActivationFunctionType.Exp,
                                                 scale=float(SCALE))
                            exv = ex[:]
                        else:
                            q0 = c * P  # live q-range is [q0, QC)
                            pss = ps3.tile([P, QC], F32, tag="s")
                            nc.tensor.matmul(
                                pss[:, q0:], kT_sb[:, kt * P : (kt + 1) * P],
                                qT_sb[:, h, q0:], start=True, stop=True,
                            )
                            nc.scalar.activation(exd_sb[:, c, q0:], pss[:, q0:],
                                                 mybir.ActivationFunctionType.Exp,
                                                 scale=float(SCALE))
                            nc.vector.tensor_tensor(
                                exd_sb[:, c, q0 : q0 + P], exd_sb[:, c, q0 : q0 + P],
                                tri_sb[:], mybir.AluOpType.mult,
                            )
                            exv = exd_sb[:, c, :]
                        nc.tensor.matmul(psD[:], ones_sb[:], exv,
                                         start=(kt == 0), stop=(kt == nkt - 1))
                        nc.tensor.matmul(psO[:], v_sb[:, kt, :], exv,
                                         start=(kt == 0), stop=(kt == nkt - 1))
                    rb = work.tile([P, QC], F32, tag="t2")
                    nc.vector.reciprocal_approx_fast(rb[:], psD[:])
                    nc.vector.tensor_tensor(oT_sb[:, h, :], psO[:], rb[:], mybir.AluOpType.mult)

                # ---- Wo partial for this chunk: out[s, hid] ----
                for st in range(QC // P):
                    for nj in range(HID // 512):
                        pswt = ps3.tile([P, QC], F32, tag="s")
                        for hk in range(HL):
                            nc.tensor.matmul(
                                pswt[:],
                                oT_sb[:, hk, st * P : (st + 1) * P],
                                wo_sb[:, hk, nj * 512 : (nj + 1) * 512],
                                start=(hk == 0), stop=(hk == HL - 1),
                            )
                        ob = outp.tile([P, 512], F32, tag="ob")
                        nc.scalar.copy(ob[:], pswt[:])
                        nc.sync.dma_start(
                            out[s0 + st * P : s0 + (st + 1) * P, nj * 512 : (nj + 1) * 512],
                            ob[:],
                        )

    nc.compile()
    return nc


_NC_CACHE = None


def _host_constants(rope_cache):
    cos = np.repeat(rope_cache[:, :, 0].T, 2, axis=0).astype(np.float32)  # [128, S]
    sin_base = np.repeat(rope_cache[:, :, 1].T, 2, axis=0).astype(np.float32)
    sign = np.where(np.arange(P) % 2 == 0, -1.0, 1.0).astype(np.float32)
    sin = sin_base * sign[:, None]
    pswap = np.zeros((P, P), np.float32)
    idx = np.arange(P)
    pswap[idx, idx ^ 1] = 1.0
    ident = np.eye(P, dtype=np.float32)
    tri = (np.arange(P)[None, :] >= np.arange(P)[:, None]).astype(np.float32)  # [k, q]
    ones = np.ones((P, P), np.float32)
    return cos, sin, pswap, ident, tri, ones


def _build_in_maps(inputs):
    x = np.asarray(inputs["x"], np.float32)
    rope_cache = np.asarray(inputs["rope_cache"], np.float32)
    Wq = np.asarray(inputs["Wq"], np.float32)
    Wk = np.asarray(inputs["Wk"], np.float32)
    Wv = np.asarray(inputs["Wv"], np.float32)
    Wo = np.asarray(inputs["Wo"], np.float32)

    cos, sin, pswap, ident, tri, ones = _host_constants(rope_cache)

    in_maps = []
    for core in range(8):
        b, tp = divmod(core, 4)
        in_maps.append({
            "xT": np.ascontiguousarray(x[b].T),
            "wq": np.ascontiguousarray(Wq[:, tp * HL * D : (tp + 1) * HL * D]),
            "wk": np.ascontiguousarray(Wk[:, tp * D : (tp + 1) * D]),
            "wv": np.ascontiguousarray(Wv[:, tp * D : (tp + 1) * D]),
            "wo": np.ascontiguousarray(Wo[tp * HL * D : (tp + 1) * HL * D, :]),
            "cosm": cos, "sinm": sin, "pswap": pswap, "ident": ident, "trim": tri,
            "onesm": ones, "zerom": np.zeros((P, QC), np.float32),
        })
    return in_maps


def kernel(x, attention_mask, rope_cache, Wq, bq, Wk, bk, Wv, bv, Wo):
    global _NC_CACHE
    in_maps = _build_in_maps({"x": x, "rope_cache": rope_cache,
                              "Wq": Wq, "Wk": Wk, "Wv": Wv, "Wo": Wo})

    if _NC_CACHE is None:
        _NC_CACHE = _build_program()
    r = run_bass_kernel_spmd(_NC_CACHE, in_maps, list(range(8)))

    outf = np.zeros((B, S, HID), np.float32)
    for core in range(8):
        b = core // 4
        outf[b] += r.results[core]["out"]
    return outf


# revision 20
# speedup vs baseline: 1.4699x; 1.0003x over previous
"""Trainium2 Bass kernel for the GQA attention layer (B=2, S=2048, HID=2048,
H=16, KVH=4, D=128, causal + RoPE).

Sharding: 8 cores = 2 (batch) x 4 (tensor-parallel over heads).
Core c handles batch b=c//4 and head group tp=c%4 (4 q-heads, 1 kv-head).
Wo is row-sharded; the 4 TP partial outputs per batch are summed on host.

v7: x is pre-transposed on the host (xT [HID, S] in DRAM) so the kernel
never runs PE transposes for the projections; all matmuls run as float32r
(full-rate fp32 PE mode) with 512-wide moving operands; causal diagonal tiles compute only the live q-range
into zero-prefixed exp buffers; reciprocal_approx_fast for the softmax
denominator. Biases bq/bk/bv are zero in the problem spec and are skipped.
The attention mask is the causal tril by construction; causality is hardcoded.
"""
import os
import sys

for p in ("/opt/trn_rl_repo", "/root/.axon_site/_ro/trn_rl_repo"):
    if os.path.isdir(p) and p not in sys.path:
        sys.path.insert(0, p)

import numpy as np

import concourse.bass as bass
import concourse.mybir as mybir
from concourse import bacc
from concourse.tile import TileContext
from concourse.bass_utils import run_bass_kernel_spmd

B, S, HID = 2, 2048, 2048
H, KVH, D = 16, 4, 128
P = 128
TP = 4                      # tensor-parallel ways
HL = H // TP                # q-heads per core (4)
QC = 512                    # q/s chunk size (fp32 moving-operand max)
NCH = S // QC               # 4 chunks
KO = HID // P               # 16 contraction chunks
DIAG = QC // P              # 4 diagonal 128-k tiles per chunk
SCALE = 1.0 / np.sqrt(D)

F32 = mybir.dt.float32
DT = mybir.dt.float32r      # attention matmul fast path


def _build_program():
    nc = bacc.Bacc(None, target_bir_lowering=False)

    xT = nc.declare_dram_parameter("xT", [HID, S], DT, isOutput=False)
    wq = nc.declare_dram_parameter("wq", [HID, HL * D], DT, isOutput=False)
    wk = nc.declare_dram_parameter("wk", [HID, D], DT, isOutput=False)
    wv = nc.declare_dram_parameter("wv", [HID, D], DT, isOutput=False)
    wo = nc.declare_dram_parameter("wo", [HL * D, HID], DT, isOutput=False)
    cosm = nc.declare_dram_parameter("cosm", [P, S], DT, isOutput=False)
    sinm = nc.declare_dram_parameter("sinm", [P, S], DT, isOutput=False)
    pswap = nc.declare_dram_parameter("pswap", [P, P], DT, isOutput=False)
    ident = nc.declare_dram_parameter("ident", [P, P], DT, isOutput=False)
    trim = nc.declare_dram_parameter("trim", [P, P], DT, isOutput=False)
    onesm = nc.declare_dram_parameter("onesm", [P, P], DT, isOutput=False)
    zerom = nc.declare_dram_parameter("zerom", [P, QC], DT, isOutput=False)
    out = nc.declare_dram_parameter("out", [S, HID], F32, isOutput=True)

    with TileContext(nc) as tc:
        with (
            tc.tile_pool(name="consts", bufs=1) as consts,
            tc.tile_pool(name="weights", bufs=1) as weights,
            tc.tile_pool(name="persist", bufs=1) as persist,
            tc.tile_pool(name="xload", bufs=1) as xload,
            tc.tile_pool(name="cspool", bufs=1) as cspool,
            tc.tile_pool(name="qop", bufs=2) as qop,
            tc.tile_pool(name="work", bufs=2) as work,
            tc.tile_pool(name="expp", bufs=6) as expp,
            tc.tile_pool(name="outp", bufs=3) as outp,
            tc.tile_pool(name="ps3", bufs=4, space="PSUM") as ps3,
            tc.tile_pool(name="psAcc", bufs=2, space="PSUM") as psAcc,
        ):
            # ---- constants ----
            pswap_sb = consts.tile([P, P], DT)
            ident_sb = consts.tile([P, P], DT)
            tri_sb = consts.tile([P, P], DT)
            ones_sb = consts.tile([P, P], DT)
            nc.sync.dma_start(ident_sb[:], ident[:])
            nc.gpsimd.dma_start(pswap_sb[:], pswap[:])
            nc.gpsimd.dma_start(tri_sb[:], trim[:])
            nc.gpsimd.dma_start(ones_sb[:], onesm[:])

            # ---- PE warm-up: keep HAM busy while the big weight DMAs land ----
            for _ in range(96):
                pwu = ps3.tile([P, P], DT, tag="s")
                nc.tensor.transpose(pwu[:], ident_sb[:], ident_sb[:])

            # ---- weights resident in SBUF ----
            wq_sb = weights.tile([P, KO, HL * D], DT)
            wk_sb = weights.tile([P, KO, D], DT)
            wv_sb = weights.tile([P, KO, D], DT)
            wo_sb = weights.tile([P, HL, HID], DT)
            nc.gpsimd.dma_start(wk_sb[:], wk.rearrange("(ko p) m -> p ko m", p=P))
            nc.gpsimd.dma_start(wv_sb[:], wv.rearrange("(ko p) m -> p ko m", p=P))
            wq_r = wq.rearrange("(ko p) m -> p ko m", p=P)
            for kg in range(4):
                nc.gpsimd.dma_start(wq_sb[:, 4 * kg : 4 * (kg + 1), :], wq_r[:, 4 * kg : 4 * (kg + 1), :])
            wo_r = wo.rearrange("(hk p) n -> p hk n", p=P)
            for ng in range(4):
                nc.gpsimd.dma_start(wo_sb[:, :, 512 * ng : 512 * (ng + 1)], wo_r[:, :, 512 * ng : 512 * (ng + 1)])

            # K^T [d, s] and V [s-part, kt, d] accumulate across chunks
            kT_sb = persist.tile([P, S], DT)
            v_sb = persist.tile([P, S // P, D], DT)

            # zero-prefixed exp buffers for the 4 diagonal tile shapes
            exd_sb = persist.tile([P, DIAG, QC], DT)
            for c in range(DIAG):
                nc.gpsimd.dma_start(exd_sb[:, c, :], zerom[:])

            xT_r = xT.rearrange("(ko p) s -> p ko s", p=P)

            for j in range(NCH):
                s0 = j * QC
                nkt = DIAG * j + DIAG  # k-tiles this chunk (incl. 4 diagonal)

                # ---- x^T chunk: straight DMA (host pre-transposed) ----
                xT_sb = xload.tile([P, KO, QC], DT)
                xeng = nc.sync if j == 0 else nc.gpsimd
                for kg in range(4):
                    xeng.dma_start(
                        xT_sb[:, 4 * kg : 4 * (kg + 1), :],
                        xT_r[:, 4 * kg : 4 * (kg + 1), s0 : s0 + QC],
                    )
                cos_sb = cspool.tile([P, QC], DT, tag="cos")
                sin_sb = cspool.tile([P, QC], DT, tag="sin")
                nc.sync.dma_start(cos_sb[:], cosm[:, s0 : s0 + QC])
                nc.sync.dma_start(sin_sb[:], sinm[:, s0 : s0 + QC])

                # ---- K projection + rope ----
                psk = ps3.tile([P, QC], F32, tag="s")
                for ko in range(KO):
                    nc.tensor.matmul(
                        psk[:], wk_sb[:, ko, :], xT_sb[:, ko, :],
                        start=(ko == 0), stop=(ko == KO - 1),
                    )
                k_raw = work.tile([P, QC], DT, tag="qraw")
                nc.scalar.copy(k_raw[:], psk[:])
                t1k = work.tile([P, QC], DT, tag="t1")
                nc.vector.tensor_tensor(t1k[:], k_raw[:], cos_sb[:], mybir.AluOpType.mult)

                # ---- V projection as V^T [d, s], then PE-transpose to [s, d] ----
                psv = ps3.tile([P, QC], F32, tag="s")
                for ko in range(KO):
                    nc.tensor.matmul(
                        psv[:], wv_sb[:, ko, :], xT_sb[:, ko, :],
                        start=(ko == 0), stop=(ko == KO - 1),
                    )
                vT_sb = work.tile([P, QC], DT, tag="qraw")
                nc.scalar.copy(vT_sb[:], psv[:])
                for st in range(QC // P):
                    psvt = ps3.tile([P, P], DT, tag="s")
                    nc.tensor.transpose(psvt[:], vT_sb[:, st * P : (st + 1) * P], ident_sb[:])
                    nc.vector.tensor_copy(v_sb[:, DIAG * j + st, :], psvt[:])

                # deferred K swap (k_raw has long been copied; no PE stall)
                pskw = ps3.tile([P, QC], F32, tag="s")
                nc.tensor.matmul(pskw[:], pswap_sb[:], k_raw[:], start=True, stop=True)
                t2k = work.tile([P, QC], DT, tag="t2")
                nc.vector.tensor_tensor(t2k[:], pskw[:], sin_sb[:], mybir.AluOpType.mult)
                nc.vector.tensor_add(kT_sb[:, s0 : s0 + QC], t1k[:], t2k[:])

                # ---- Q projection + rope (4 heads) ----
                qT_sb = qop.tile([P, HL, QC], DT, tag="qT")

                def emit_qswap(hh, q_raw_h, t1_h):
                    pssw = ps3.tile([P, QC], F32, tag="s")
                    nc.tensor.matmul(pssw[:], pswap_sb[:], q_raw_h[:], start=True, stop=True)
                    t2 = work.tile([P, QC], DT, tag="t2")
                    nc.vector.tensor_tensor(t2[:], pssw[:], sin_sb[:], mybir.AluOpType.mult)
                    nc.vector.tensor_add(qT_sb[:, hh, :], t1_h[:], t2[:])

                pending = None
                for h in range(HL):
                    psq = ps3.tile([P, QC], F32, tag="s")
                    for ko in range(KO):
                        nc.tensor.matmul(
                            psq[:], wq_sb[:, ko, h * D : (h + 1) * D], xT_sb[:, ko, :],
                            start=(ko == 0), stop=(ko == KO - 1),
                        )
                    if pending is not None:
                        emit_qswap(*pending)
                    q_raw = work.tile([P, QC], DT, tag="qraw")
                    nc.scalar.copy(q_raw[:], psq[:])
                    t1 = work.tile([P, QC], DT, tag="t1")
                    nc.vector.tensor_tensor(t1[:], q_raw[:], cos_sb[:], mybir.AluOpType.mult)
                    pending = (h, q_raw, t1)
                emit_qswap(*pending)

                # ---- attention for this q-chunk (scores transposed [k, q]) ----
                oT_sb = qop.tile([P, HL, QC], DT, tag="oT")
                for h in range(HL):
                    psO = psAcc.tile([P, QC], F32, tag="O")
                    psD = psAcc.tile([P, QC], F32, tag="D")
                    for kt in range(nkt):
                        c = kt - DIAG * j  # >= 0 on the diagonal band
                        if c < 0:
                            pss = ps3.tile([P, QC], F32, tag="s")
                            nc.tensor.matmul(
                                pss[:], kT_sb[:, kt * P : (kt + 1) * P], qT_sb[:, h, :],
                                start=True, stop=True,
                            )
                            ex = expp.tile([P, QC], DT, tag="ex")
                            nc.scalar.activation(ex[:], pss[:], mybir.ActivationFunctionType.Exp,
                                                 scale=float(SCALE))
                            exv = ex[:]
                        else:
                            q0 = c * P  # live q-range is [q0, QC)
                            pss = ps3.tile([P, QC], F32, tag="s")
                            nc.tensor.matmul(
                                pss[:, q0:], kT_sb[:, kt * P : (kt + 1) * P],
                                qT_sb[:, h, q0:], start=True, stop=True,
                            )
                            nc.scalar.activation(exd_sb[:, c, q0:], pss[:, q0:],
                                                 mybir.ActivationFunctionType.Exp,
                                                 scale=float(SCALE))
                            nc.vector.tensor_tensor(
                                exd_sb[:, c, q0 : q0 + P], exd_sb[:, c, q0 : q0 + P],
                                tri_sb[:], mybir.AluOpType.mult,
                            )
                            exv = exd_sb[:, c, :]
                        nc.tensor.matmul(psD[:], ones_sb[:], exv,
                                         start=(kt == 0), stop=(kt == nkt - 1))
                        nc.tensor.matmul(psO[:], v_sb[:, kt, :], exv,
                                         start=(kt == 0), stop=(kt == nkt - 1))
                    rb = work.tile([P, QC], F32, tag="t2")
                    nc.vector.reciprocal_approx_fast(rb[:], psD[:])
                    nc.vector.tensor_tensor(oT_sb[:, h, :], psO[:], rb[:], mybir.AluOpType.mult)

                # ---- Wo partial for this chunk: out[s, hid] ----
                for st in range(QC // P):
                    for nj in range(HID // 512):
                        pswt = ps3.tile([P, QC], F32, tag="s")
                        for hk in range(HL):
                            nc.tensor.matmul(
                                pswt[:],
                                oT_sb[:, hk, st * P : (st + 1) * P],
                                wo_sb[:, hk, nj * 512 : (nj + 1) * 512],
                                start=(hk == 0), stop=(hk == HL - 1),
                            )
                        ob = outp.tile([P, 512], F32, tag="ob")
                        nc.scalar.copy(ob[:], pswt[:])
                        nc.sync.dma_start(
                            out[s0 + st * P : s0 + (st + 1) * P, nj * 512 : (nj + 1) * 512],
                            ob[:],
                        )

    nc.compile()
    return nc


_NC_CACHE = None


def _host_constants(rope_cache):
    cos = np.repeat(rope_cache[:, :, 0].T, 2, axis=0).astype(np.float32)  # [128, S]
    sin_base = np.repeat(rope_cache[:, :, 1].T, 2, axis=0).astype(np.float32)
    sign = np.where(np.arange(P) % 2 == 0, -1.0, 1.0).astype(np.float32)
    sin = sin_base * sign[:, None]
    pswap = np.zeros((P, P), np.float32)
    idx = np.arange(P)
    pswap[idx, idx ^ 1] = 1.0
    ident = np.eye(P, dtype=np.float32)
    tri = (np.arange(P)[None, :] >= np.arange(P)[:, None]).astype(np.float32)  # [k, q]
    ones = np.ones((P, P), np.float32)
    return cos, sin, pswap, ident, tri, ones


def _build_in_maps(inputs):
    x = np.asarray(inputs["x"], np.float32)
    rope_cache = np.asarray(inputs["rope_cache"], np.float32)
    Wq = np.asarray(inputs["Wq"], np.float32)
    Wk = np.asarray(inputs["Wk"], np.float32)
    Wv = np.asarray(inputs["Wv"], np.float32)
    Wo = np.asarray(inputs["Wo"], np.float32)

    cos, sin, pswap, ident, tri, ones = _host_constants(rope_cache)

    in_maps = []
    for core in range(8):
        b, tp = divmod(core, 4)
        in_maps.append({
            "xT": np.ascontiguousarray(x[b].T),
            "wq": np.ascontiguousarray(Wq[:, tp * HL * D : (tp + 1) * HL * D]),
            "wk": np.ascontiguousarray(Wk[:, tp * D : (tp + 1) * D]),
            "wv": np.ascontiguousarray(Wv[:, tp * D : (tp + 1) * D]),
            "wo": np.ascontiguousarray(Wo[tp * HL * D : (tp + 1) * HL * D, :]),
            "cosm": cos, "sinm": sin, "pswap": pswap, "ident": ident, "trim": tri,
            "onesm": ones, "zerom": np.zeros((P, QC), np.float32),
        })
    return in_maps


def kernel(x, attention_mask, rope_cache, Wq, bq, Wk, bk, Wv, bv, Wo):
    global _NC_CACHE
    in_maps = _build_in_maps({"x": x, "rope_cache": rope_cache,
                              "Wq": Wq, "Wk": Wk, "Wv": Wv, "Wo": Wo})

    if _NC_CACHE is None:
        _NC_CACHE = _build_program()
    r = run_bass_kernel_spmd(_NC_CACHE, in_maps, list(range(8)))

    outf = np.zeros((B, S, HID), np.float32)
    for core in range(8):
        b = core // 4
        outf[b] += r.results[core]["out"]
    return outf


# revision 23
# speedup vs baseline: 1.4879x; 1.0123x over previous
"""Trainium2 Bass kernel for the GQA attention layer (B=2, S=2048, HID=2048,
H=16, KVH=4, D=128, causal + RoPE).

Sharding: 8 cores = 2 (batch) x 4 (tensor-parallel over heads).
Core c handles batch b=c//4 and head group tp=c%4 (4 q-heads, 1 kv-head).
Wo is row-sharded; the 4 TP partial outputs per batch are summed on host.

x is pre-transposed on the host (xT [HID, S] in DRAM) so the kernel never
runs PE transposes for the projections; all matmuls run as float32r
(full-rate fp32 PE mode) with 512-wide moving operands; causal diagonal
tiles compute only the live q-range into zero-prefixed exp buffers;
reciprocal_approx_fast for the softmax denominator; rope swap matmuls are
emitted one head late so they never stall the PE stream; x-chunk loads for
chunks 1-3 ride the gpsimd DMA rail so output tiles drain promptly on sync. Biases bq/bk/bv are zero in the problem spec and are skipped.
The attention mask is the causal tril by construction; causality is hardcoded.
"""
import os
import sys

for p in ("/opt/trn_rl_repo", "/root/.axon_site/_ro/trn_rl_repo"):
    if os.path.isdir(p) and p not in sys.path:
        sys.path.insert(0, p)

import numpy as np

import concourse.bass as bass
import concourse.mybir as mybir
from concourse import bacc
from concourse.tile import TileContext
from concourse.bass_utils import run_bass_kernel_spmd

B, S, HID = 2, 2048, 2048
H, KVH, D = 16, 4, 128
P = 128
TP = 4                      # tensor-parallel ways
HL = H // TP                # q-heads per core (4)
QC = 512                    # q/s chunk size (fp32 moving-operand max)
NCH = S // QC               # 4 chunks
KO = HID // P               # 16 contraction chunks
DIAG = QC // P              # 4 diagonal 128-k tiles per chunk
SCALE = 1.0 / np.sqrt(D)

F32 = mybir.dt.float32
DT = mybir.dt.float32r      # attention matmul fast path


def _build_program():
    nc = bacc.Bacc(None, target_bir_lowering=False)

    xT = nc.declare_dram_parameter("xT", [HID, S], DT, isOutput=False)
    wq = nc.declare_dram_parameter("wq", [HID, HL * D], DT, isOutput=False)
    wk = nc.declare_dram_parameter("wk", [HID, D], DT, isOutput=False)
    wv = nc.declare_dram_parameter("wv", [HID, D], DT, isOutput=False)
    wo = nc.declare_dram_parameter("wo", [HL * D, HID], DT, isOutput=False)
    cosm = nc.declare_dram_parameter("cosm", [P, S], DT, isOutput=False)
    sinm = nc.declare_dram_parameter("sinm", [P, S], DT, isOutput=False)
    pswap = nc.declare_dram_parameter("pswap", [P, P], DT, isOutput=False)
    ident = nc.declare_dram_parameter("ident", [P, P], DT, isOutput=False)
    trim = nc.declare_dram_parameter("trim", [P, P], DT, isOutput=False)
    onesm = nc.declare_dram_parameter("onesm", [P, P], DT, isOutput=False)
    zerom = nc.declare_dram_parameter("zerom", [P, QC], DT, isOutput=False)
    out = nc.declare_dram_parameter("out", [S, HID], F32, isOutput=True)

    with TileContext(nc) as tc:
        with (
            tc.tile_pool(name="consts", bufs=1) as consts,
            tc.tile_pool(name="weights", bufs=1) as weights,
            tc.tile_pool(name="persist", bufs=1) as persist,
            tc.tile_pool(name="xload", bufs=1) as xload,
            tc.tile_pool(name="cspool", bufs=1) as cspool,
            tc.tile_pool(name="qop", bufs=2) as qop,
            tc.tile_pool(name="work", bufs=2) as work,
            tc.tile_pool(name="expp", bufs=6) as expp,
            tc.tile_pool(name="outp", bufs=3) as outp,
            tc.tile_pool(name="ps3", bufs=4, space="PSUM") as ps3,
            tc.tile_pool(name="psAcc", bufs=2, space="PSUM") as psAcc,
        ):
            # ---- constants ----
            pswap_sb = consts.tile([P, P], DT)
            ident_sb = consts.tile([P, P], DT)
            tri_sb = consts.tile([P, P], DT)
            ones_sb = consts.tile([P, P], DT)
            nc.sync.dma_start(ident_sb[:], ident[:])
            nc.gpsimd.dma_start(pswap_sb[:], pswap[:])
            nc.gpsimd.dma_start(tri_sb[:], trim[:])
            nc.gpsimd.dma_start(ones_sb[:], onesm[:])

            # ---- PE warm-up: keep HAM busy while the big weight DMAs land ----
            for _ in range(96):
                pwu = ps3.tile([P, P], DT, tag="s")
                nc.tensor.transpose(pwu[:], ident_sb[:], ident_sb[:])

            # ---- weights resident in SBUF ----
            wq_sb = weights.tile([P, KO, HL * D], DT)
            wk_sb = weights.tile([P, KO, D], DT)
            wv_sb = weights.tile([P, KO, D], DT)
            wo_sb = weights.tile([P, HL, HID], DT)
            nc.gpsimd.dma_start(wk_sb[:], wk.rearrange("(ko p) m -> p ko m", p=P))
            nc.gpsimd.dma_start(wv_sb[:], wv.rearrange("(ko p) m -> p ko m", p=P))
            xT_r0 = xT.rearrange("(ko p) s -> p ko s", p=P)
            xT0_sb = xload.tile([P, KO, QC], DT, name="xT0_sb", tag="xT_sb")
            for kg in range(4):
                eng0 = nc.sync if kg < 2 else nc.gpsimd
                eng0.dma_start(xT0_sb[:, 4 * kg : 4 * (kg + 1), :],
                               xT_r0[:, 4 * kg : 4 * (kg + 1), 0:QC])
            wq_r = wq.rearrange("(ko p) m -> p ko m", p=P)
            for kg in range(4):
                nc.gpsimd.dma_start(wq_sb[:, 4 * kg : 4 * (kg + 1), :], wq_r[:, 4 * kg : 4 * (kg + 1), :])
            wo_r = wo.rearrange("(hk p) n -> p hk n", p=P)
            for ng in range(4):
                nc.gpsimd.dma_start(wo_sb[:, :, 512 * ng : 512 * (ng + 1)], wo_r[:, :, 512 * ng : 512 * (ng + 1)])

            # K^T [d, s] and V [s-part, kt, d] accumulate across chunks
            kT_sb = persist.tile([P, S], DT)
            v_sb = persist.tile([P, S // P, D], DT)

            # zero-prefixed exp buffers for the 4 diagonal tile shapes
            exd_sb = persist.tile([P, DIAG, QC], DT)
            for c in range(DIAG):
                nc.gpsimd.dma_start(exd_sb[:, c, :], zerom[:])

            xT_r = xT.rearrange("(ko p) s -> p ko s", p=P)

            for j in range(NCH):
                s0 = j * QC
                nkt = DIAG * j + DIAG  # k-tiles this chunk (incl. 4 diagonal)

                # ---- x^T chunk: straight DMA (host pre-transposed) ----
                if j == 0:
                    xT_sb = xT0_sb  # preloaded during weight DMAs
                else:
                    xT_sb = xload.tile([P, KO, QC], DT, tag="xT_sb")
                    for kg in range(4):
                        nc.gpsimd.dma_start(
                            xT_sb[:, 4 * kg : 4 * (kg + 1), :],
                            xT_r[:, 4 * kg : 4 * (kg + 1), s0 : s0 + QC],
                        )
                cos_sb = cspool.tile([P, QC], DT, tag="cos")
                sin_sb = cspool.tile([P, QC], DT, tag="sin")
                nc.sync.dma_start(cos_sb[:], cosm[:, s0 : s0 + QC])
                nc.sync.dma_start(sin_sb[:], sinm[:, s0 : s0 + QC])

                # ---- K projection + rope ----
                psk = ps3.tile([P, QC], F32, tag="s")
                for ko in range(KO):
                    nc.tensor.matmul(
                        psk[:], wk_sb[:, ko, :], xT_sb[:, ko, :],
                        start=(ko == 0), stop=(ko == KO - 1),
                    )
                k_raw = work.tile([P, QC], DT, tag="qraw")
                nc.scalar.copy(k_raw[:], psk[:])
                t1k = work.tile([P, QC], DT, tag="t1")
                nc.vector.tensor_tensor(t1k[:], k_raw[:], cos_sb[:], mybir.AluOpType.mult)

                # ---- V projection as V^T [d, s], then PE-transpose to [s, d] ----
                psv = ps3.tile([P, QC], F32, tag="s")
                for ko in range(KO):
                    nc.tensor.matmul(
                        psv[:], wv_sb[:, ko, :], xT_sb[:, ko, :],
                        start=(ko == 0), stop=(ko == KO - 1),
                    )
                vT_sb = work.tile([P, QC], DT, tag="qraw")
                nc.scalar.copy(vT_sb[:], psv[:])
                for st in range(QC // P):
                    psvt = ps3.tile([P, P], DT, tag="s")
                    nc.tensor.transpose(psvt[:], vT_sb[:, st * P : (st + 1) * P], ident_sb[:])
                    nc.vector.tensor_copy(v_sb[:, DIAG * j + st, :], psvt[:])

                # deferred K swap (k_raw has long been copied; no PE stall)
                pskw = ps3.tile([P, QC], F32, tag="s")
                nc.tensor.matmul(pskw[:], pswap_sb[:], k_raw[:], start=True, stop=True)
                t2k = work.tile([P, QC], DT, tag="t2")
                nc.vector.tensor_tensor(t2k[:], pskw[:], sin_sb[:], mybir.AluOpType.mult)
                nc.vector.tensor_add(kT_sb[:, s0 : s0 + QC], t1k[:], t2k[:])

                # ---- Q projection + rope (4 heads) ----
                qT_sb = qop.tile([P, HL, QC], DT, tag="qT")

                def emit_qswap(hh, q_raw_h, t1_h):
                    pssw = ps3.tile([P, QC], F32, tag="s")
                    nc.tensor.matmul(pssw[:], pswap_sb[:], q_raw_h[:], start=True, stop=True)
                    t2 = work.tile([P, QC], DT, tag="t2")
                    nc.vector.tensor_tensor(t2[:], pssw[:], sin_sb[:], mybir.AluOpType.mult)
                    nc.vector.tensor_add(qT_sb[:, hh, :], t1_h[:], t2[:])

                pending = None
                for h in range(HL):
                    psq = ps3.tile([P, QC], F32, tag="s")
                    for ko in range(KO):
                        nc.tensor.matmul(
                            psq[:], wq_sb[:, ko, h * D : (h + 1) * D], xT_sb[:, ko, :],
                            start=(ko == 0), stop=(ko == KO - 1),
                        )
                    if pending is not None:
                        emit_qswap(*pending)
                    q_raw = work.tile([P, QC], DT, tag="qraw")
                    nc.scalar.copy(q_raw[:], psq[:])
                    t1 = work.tile([P, QC], DT, tag="t1")
                    nc.vector.tensor_tensor(t1[:], q_raw[:], cos_sb[:], mybir.AluOpType.mult)
                    pending = (h, q_raw, t1)
                emit_qswap(*pending)

                # ---- attention for this q-chunk (scores transposed [k, q]) ----
                oT_sb = qop.tile([P, HL, QC], DT, tag="oT")
                for h in range(HL):
                    psO = psAcc.tile([P, QC], F32, tag="O")
                    psD = psAcc.tile([P, QC], F32, tag="D")
                    for kt in range(nkt):
                        c = kt - DIAG * j  # >= 0 on the diagonal band
                        if c < 0:
                            pss = ps3.tile([P, QC], F32, tag="s")
                            nc.tensor.matmul(
                                pss[:], kT_sb[:, kt * P : (kt + 1) * P], qT_sb[:, h, :],
                                start=True, stop=True,
                            )
                            ex = expp.tile([P, QC], DT, tag="ex")
                            nc.scalar.activation(ex[:], pss[:], mybir.ActivationFunctionType.Exp,
                                                 scale=float(SCALE))
                            exv = ex[:]
                        else:
                            q0 = c * P  # live q-range is [q0, QC)
                            pss = ps3.tile([P, QC], F32, tag="s")
                            nc.tensor.matmul(
                                pss[:, q0:], kT_sb[:, kt * P : (kt + 1) * P],
                                qT_sb[:, h, q0:], start=True, stop=True,
                            )
                            nc.scalar.activation(exd_sb[:, c, q0:], pss[:, q0:],
                                                 mybir.ActivationFunctionType.Exp,
                                                 scale=float(SCALE))
                            nc.vector.tensor_tensor(
                                exd_sb[:, c, q0 : q0 + P], exd_sb[:, c, q0 : q0 + P],
                                tri_sb[:], mybir.AluOpType.mult,
                            )
                            exv = exd_sb[:, c, :]
                        nc.tensor.matmul(psD[:], ones_sb[:], exv,
                                         start=(kt == 0), stop=(kt == nkt - 1))
                        nc.tensor.matmul(psO[:], v_sb[:, kt, :], exv,
                                         start=(kt == 0), stop=(kt == nkt - 1))
                    rb = work.tile([P, QC], F32, tag="t2")
                    nc.vector.reciprocal_approx_fast(rb[:], psD[:])
                    nc.vector.tensor_tensor(oT_sb[:, h, :], psO[:], rb[:], mybir.AluOpType.mult)

                # ---- Wo partial for this chunk: out[s, hid] ----
                for st in range(QC // P):
                    for nj in range(HID // 512):
                        pswt = ps3.tile([P, QC], F32, tag="s")
                        for hk in range(HL):
                            nc.tensor.matmul(
                                pswt[:],
                                oT_sb[:, hk, st * P : (st + 1) * P],
                                wo_sb[:, hk, nj * 512 : (nj + 1) * 512],
                                start=(hk == 0), stop=(hk == HL - 1),
                            )
                        ob = outp.tile([P, 512], F32, tag="ob")
                        nc.scalar.copy(ob[:], pswt[:])
                        nc.sync.dma_start(
                            out[s0 + st * P : s0 + (st + 1) * P, nj * 512 : (nj + 1) * 512],
                            ob[:],
                        )

    nc.compile()
    return nc


_NC_CACHE = None


def _host_constants(rope_cache):
    cos = np.repeat(rope_cache[:, :, 0].T, 2, axis=0).astype(np.float32)  # [128, S]
    sin_base = np.repeat(rope_cache[:, :, 1].T, 2, axis=0).astype(np.float32)
    sign = np.where(np.arange(P) % 2 == 0, -1.0, 1.0).astype(np.float32)
    sin = sin_base * sign[:, None]
    pswap = np.zeros((P, P), np.float32)
    idx = np.arange(P)
    pswap[idx, idx ^ 1] = 1.0
    ident = np.eye(P, dtype=np.float32)
    tri = (np.arange(P)[None, :] >= np.arange(P)[:, None]).astype(np.float32)  # [k, q]
    ones = np.ones((P, P), np.float32)
    return cos, sin, pswap, ident, tri, ones


def _build_in_maps(inputs):
    x = np.asarray(inputs["x"], np.float32)
    rope_cache = np.asarray(inputs["rope_cache"], np.float32)
    Wq = np.asarray(inputs["Wq"], np.float32)
    Wk = np.asarray(inputs["Wk"], np.float32)
    Wv = np.asarray(inputs["Wv"], np.float32)
    Wo = np.asarray(inputs["Wo"], np.float32)

    cos, sin, pswap, ident, tri, ones = _host_constants(rope_cache)

    in_maps = []
    for core in range(8):
        b, tp = divmod(core, 4)
        in_maps.append({
            "xT": np.ascontiguousarray(x[b].T),
            "wq": np.ascontiguousarray(Wq[:, tp * HL * D : (tp + 1) * HL * D]),
            "wk": np.ascontiguousarray(Wk[:, tp * D : (tp + 1) * D]),
            "wv": np.ascontiguousarray(Wv[:, tp * D : (tp + 1) * D]),
            "wo": np.ascontiguousarray(Wo[tp * HL * D : (tp + 1) * HL * D, :]),
            "cosm": cos, "sinm": sin, "pswap": pswap, "ident": ident, "trim": tri,
            "onesm": ones, "zerom": np.zeros((P, QC), np.float32),
        })
    return in_maps


def kernel(x, attention_mask, rope_cache, Wq, bq, Wk, bk, Wv, bv, Wo):
    global _NC_CACHE
    in_maps = _build_in_maps({"x": x, "rope_cache": rope_cache,
                              "Wq": Wq, "Wk": Wk, "Wv": Wv, "Wo": Wo})

    if _NC_CACHE is None:
        _NC_CACHE = _build_program()
    r = run_bass_kernel_spmd(_NC_CACHE, in_maps, list(range(8)))

    outf = np.zeros((B, S, HID), np.float32)
    for core in range(8):
        b = core // 4
        outf[b] += r.results[core]["out"]
    return outf
